# revision 22
# baseline (speedup 1.0000x reference)
"""Trainium2 Bass kernel for MergedColumnParallelLinearWithLoRA.

Computes  out = x @ W.T + concat(lora1(x), lora2(x))  where
lora_i(x)[t] = B_i[l_t] @ (A_i[l_t] @ x[t]) + bias_i[l_t],  l_t = indices[t].

Sharding: ROW-parallel (token-sharded) across 8 NeuronCores. Core c owns
tokens [c*1024, (c+1)*1024); x and indices are sharded along tokens, W /
lora weights are used in full by every core (streamed from HBM). This makes
the LoRA shrink naturally local (no replicated work, no collectives): each
core only computes s = A @ x_t for its own tokens.

Per-core device program:
  - x^T resident in SBUF ([128, 8, 16, 128], d-major tiles).
  - Augmented weight W_aug = [A1_flat; A2_flat; W] as 23 column-chunks of 512.
  - Chunk 0 = LoRA shrink: s1|s2 per token tile -> masked dispatch
    (s_masked = s * (lora_id_col == idx)), one-hot oh = (iota16 == idx),
    PE-transposed into resident s^T / oh^T tiles.
  - Chunks 1..22 = base GEMM, streamed W; the LoRA expand + bias
    ( y = [s_masked | oh] @ [B_flat_chunk; bias_chunk] ) accumulates into the
    same PSUM bank right after the 16 base k-matmuls (start=False).
  - All matmuls fp32r (full PE rate, ~1.4e-4 rel precision), N=512 uniform.
  - PSUM -> SBUF copies split across ScalarE/VectorE, then DMA out.
"""

import numpy as np

import concourse.bass as bass  # noqa: F401
import concourse.mybir as mybir
import concourse.tile as tile
from concourse import bacc
from concourse.masks import make_identity

T, D, O, L, R = 8192, 2048, 5632, 16, 16
NCORES = 8
TL = T // NCORES  # 1024 tokens per core
P = 128
KT = D // P  # 16 k-tiles
MTL = TL // P  # 8 local token tiles
SH = 2 * L * R  # 512 shrink columns (s1 | s2)
NF = 2 * O  # 11264 full output columns
NCH = NF // 512  # 22 base chunks
WA = SH + NF  # 11776 augmented columns = 23 chunks of 512
F32 = mybir.dt.float32
F32R = mybir.dt.float32r
I32 = mybir.dt.int32


def build_nc(reps=1, mode="full", bias_via="dma"):
    """mode: 'full' | 'base' (no LoRA shrink/expand).
    bias_via: 'dma' (indirect-DMA gather + vector add, assumes indices>=0)
              | 'pe' (one-hot K=16 matmul on the PE).
    """
    nc = bacc.Bacc("TRN2", target_bir_lowering=False, debug=False)

    xt = nc.dram_tensor("xt", [MTL, P, KT, P], F32, kind="ExternalInput")
    wt = nc.dram_tensor("wt", [NCH + 1, P, KT, 512], F32, kind="ExternalInput")
    b1 = nc.dram_tensor("b1", [2 * P + L, O], F32, kind="ExternalInput")
    b2 = nc.dram_tensor("b2", [2 * P + L, O], F32, kind="ExternalInput")
    c1 = nc.dram_tensor("c1", [L, O], F32, kind="ExternalInput")
    c2 = nc.dram_tensor("c2", [L, O], F32, kind="ExternalInput")
    idx = nc.dram_tensor("idx", [P, MTL], I32, kind="ExternalInput")
    out = nc.dram_tensor("out", [TL, NF], F32, kind="ExternalOutput")

    bdram = (b1, b2)
    cdram = (c1, c2)

    with tile.TileContext(nc) as tc:
        with (
            tc.tile_pool(name="const", bufs=1) as const,
            tc.tile_pool(name="wpool", bufs=2) as wpool,
            tc.tile_pool(name="bpool", bufs=2) as bpool,
            tc.tile_pool(name="spool", bufs=3) as spool,
            tc.tile_pool(name="opool", bufs=4) as opool,
            tc.tile_pool(name="gpool", bufs=4) as gpool,
            tc.tile_pool(name="ps_b", bufs=8, space="PSUM") as ps_b,
        ):
            # ---------------- resident constants ----------------
            # startup-critical DMAs first, k-sliced so the first shrink
            # matmuls only wait on their own k-slice
            t_xr = const.tile([P, MTL, KT, P], F32R, tag="xr", name="t_xr")
            t_w0 = wpool.tile([P, KT, 512], F32R, tag="w", name="t_w0")
            for kk in range(KT):
                nc.sync.dma_start(t_xr[:, 0, kk], xt[0, :, kk].bitcast(F32R))
                nc.sync.dma_start(t_w0[:, kk], wt[0, :, kk].bitcast(F32R))
            for mtl in range(1, MTL):
                nc.sync.dma_start(t_xr[:, mtl], xt[mtl].bitcast(F32R))

            t_idx = const.tile([P, MTL], I32, tag="idxi", name="t_idx")
            nc.sync.dma_start(t_idx[:], idx[:])
            t_idxf = const.tile([P, MTL], F32, tag="idxf", name="t_idxf")
            nc.vector.tensor_copy(t_idxf[:], t_idx[:])

            t_identf = const.tile([P, P], F32, tag="identf", name="t_identf")
            make_identity(nc, t_identf[:])
            t_ident = const.tile([P, P], F32R, tag="ident", name="t_ident")
            nc.vector.tensor_copy(t_ident[:], t_identf[:])

            # lora-id per shrink column: col j (within s1 or s2) -> j // R
            t_lidi = const.tile([P, 2, L, R], I32, tag="lidi", name="t_lidi")
            nc.gpsimd.iota(
                t_lidi[:], pattern=[[0, 2], [1, L], [0, R]], base=0, channel_multiplier=0
            )
            t_lid = const.tile([P, SH], F32, tag="lid", name="t_lid")
            nc.vector.tensor_copy(t_lid[:], t_lidi[:].rearrange("p a l r -> p (a l r)"))

            t_i16i = const.tile([P, L], I32, tag="i16i", name="t_i16i")
            nc.gpsimd.iota(t_i16i[:], pattern=[[1, L]], base=0, channel_multiplier=0)
            t_i16 = const.tile([P, L], F32, tag="i16", name="t_i16")
            nc.vector.tensor_copy(t_i16[:], t_i16i[:])

            # resident transposed masked-shrink + one-hot
            t_st = const.tile([P, MTL, 4 * P], F32R, tag="st", name="t_st")
            t_oh = const.tile([L, MTL, P], F32R, tag="oh", name="t_oh")

            for _rep in range(reps):
                # ---------------- chunk 0: LoRA shrink ----------------
                if mode == "full":
                    pend = []

                    def _transpose_sa(mtl, t_sa):
                        p_t = ps_b.tile([P, 4 * P], F32R, tag="b", name="p_t")
                        for j in range(4):
                            nc.tensor.transpose(
                                p_t[:, j * P : (j + 1) * P],
                                t_sa[:, j * P : (j + 1) * P],
                                t_ident[:],
                            )
                        nc.vector.tensor_copy(t_st[:, mtl, :], p_t[:])
                        if bias_via == "pe":
                            p_to = ps_b.tile([L, P], F32R, tag="b", name="p_to")
                            nc.tensor.transpose(
                                p_to[:], t_sa[:, SH : SH + L], t_ident[:]
                            )
                            nc.vector.tensor_copy(t_oh[:, mtl, :], p_to[:])

                    if _rep > 0:
                        t_w0 = wpool.tile([P, KT, 512], F32R, tag="w", name="t_w0")
                        nc.sync.dma_start(t_w0[:], wt[0].bitcast(F32R))
                    for mtl in range(MTL):
                        p_s = ps_b.tile([P, SH], F32, tag="b", name="p_s")
                        for kk in range(KT):
                            nc.tensor.matmul(
                                p_s[:],
                                t_xr[:, mtl, kk, :],
                                t_w0[:, kk, :],
                                start=(kk == 0),
                                stop=(kk == KT - 1),
                            )
                        idx_ap = t_idxf[:, mtl : mtl + 1]
                        t_sa = spool.tile([P, SH + L], F32R, tag="sa", name="t_sa")
                        nc.vector.scalar_tensor_tensor(
                            t_sa[:, 0:SH],
                            t_lid[:],
                            idx_ap,
                            p_s[:],
                            op0=mybir.AluOpType.is_equal,
                            op1=mybir.AluOpType.mult,
                        )
                        if bias_via == "pe":
                            nc.vector.tensor_scalar(
                                t_sa[:, SH : SH + L],
                                t_i16[:],
                                idx_ap,
                                None,
                                op0=mybir.AluOpType.is_equal,
                            )
                        pend.append((mtl, t_sa))
                        if len(pend) >= 2:
                            _transpose_sa(*pend.pop(0))
                    while pend:
                        _transpose_sa(*pend.pop(0))

                # ---------------- chunks 1..22: base + expand ----------------
                for ch in range(1, NCH + 1):
                    s, ci = divmod(ch - 1, NCH // 2)
                    t_wc = wpool.tile([P, KT, 512], F32R, tag="w", name="t_wc")
                    for kk in range(KT):
                        nc.sync.dma_start(t_wc[:, kk], wt[ch, :, kk].bitcast(F32R))
                    if mode == "full":
                        t_b = bpool.tile([P, 2, 512], F32R, tag="bb", name="t_b")
                        nc.sync.dma_start(
                            t_b[:],
                            bdram[s][0 : 2 * P, ci * 512 : (ci + 1) * 512]
                            .rearrange("(c p) o -> p c o", p=P)
                            .bitcast(F32R),
                        )
                        if bias_via == "pe":
                            t_bb = bpool.tile([L, 512], F32R, tag="bc", name="t_bb")
                            nc.sync.dma_start(
                                t_bb[:],
                                bdram[s][
                                    2 * P : 2 * P + L, ci * 512 : (ci + 1) * 512
                                ].bitcast(F32R),
                            )
                    for mtl in range(MTL):
                        p_b = ps_b.tile([P, 512], F32, tag="b", name="p_b")
                        do_exp = mode == "full"
                        for kk in range(KT):
                            nc.tensor.matmul(
                                p_b[:],
                                t_xr[:, mtl, kk, :],
                                t_wc[:, kk, :],
                                start=(kk == 0),
                                stop=(not do_exp and kk == KT - 1),
                            )
                        if do_exp:
                            for c in range(2):
                                nc.tensor.matmul(
                                    p_b[:],
                                    t_st[:, mtl, (2 * s + c) * P : (2 * s + c + 1) * P],
                                    t_b[:, c, :],
                                    start=False,
                                    stop=(bias_via != "pe" and c == 1),
                                )
                            if bias_via == "pe":
                                nc.tensor.matmul(
                                    p_b[:],
                                    t_oh[:, mtl, :],
                                    t_bb[:],
                                    start=False,
                                    stop=True,
                                )
                        t_out = opool.tile([P, 512], F32, tag="o", name="t_out")
                        if do_exp and bias_via == "dma":
                            t_bg = gpool.tile([P, 512], F32, tag="g", name="t_bg")
                            nc.gpsimd.indirect_dma_start(
                                out=t_bg[:],
                                out_offset=None,
                                in_=cdram[s][:],
                                in_offset=bass.IndirectOffsetOnAxis(
                                    ap=t_idx[:, mtl : mtl + 1], axis=0
                                ),
                                element_offset=ci * 512,
                            )
                            nc.vector.tensor_tensor(
                                t_out[:], p_b[:], t_bg[:], op=mybir.AluOpType.add
                            )
                        elif (ch + mtl) % 2 == 0:
                            nc.vector.tensor_copy(t_out[:], p_b[:])
                        else:
                            nc.scalar.copy(t_out[:], p_b[:])
                        nc.sync.dma_start(
                            out[
                                mtl * P : (mtl + 1) * P,
                                (ch - 1) * 512 : ch * 512,
                            ],
                            t_out[:],
                        )

    nc.compile()
    return nc


# ---------------------------------------------------------------------------
# host-side sharding / unsharding
# ---------------------------------------------------------------------------


def shard_inputs(x, W, lora_a1, lora_a2, lora_b1, lora_b2, bias1, bias2, indices):
    x = np.asarray(x, np.float32)
    W = np.asarray(W, np.float32)
    indices = np.asarray(indices, np.int32)

    a1f = np.asarray(lora_a1, np.float32).reshape(L * R, D)
    a2f = np.asarray(lora_a2, np.float32).reshape(L * R, D)
    w_aug = np.concatenate([a1f, a2f, W], axis=0)  # [11776, 2048]
    # wt[ch, p, kk, j] = w_aug[ch*512 + j, kk*128 + p]
    wt = np.ascontiguousarray(
        w_aug.T.reshape(KT, P, NCH + 1, 512).transpose(2, 1, 0, 3)
    )

    def bmat(lb, bias):
        bf = np.asarray(lb, np.float32).transpose(0, 2, 1).reshape(L * R, O)
        return np.ascontiguousarray(
            np.concatenate([bf, np.asarray(bias, np.float32)], axis=0)
        )

    b1m = bmat(lora_b1, bias1)
    b2m = bmat(lora_b2, bias2)

    # xt[c][mtl, p, kk, m] = x[c*1024 + mtl*128 + m, kk*128 + p]
    xts = x.reshape(NCORES, MTL, P, KT, P).transpose(0, 1, 4, 3, 2)
    idxs = indices.reshape(NCORES, MTL, P).transpose(0, 2, 1)

    in_maps = []
    for c in range(NCORES):
        in_maps.append(
            {
                "xt": np.ascontiguousarray(xts[c]),
                "wt": wt,
                "b1": b1m,
                "b2": b2m,
                "c1": np.ascontiguousarray(np.asarray(bias1, np.float32)),
                "c2": np.ascontiguousarray(np.asarray(bias2, np.float32)),
                "idx": np.ascontiguousarray(idxs[c]),
            }
        )
    return in_maps


def unshard_output(results):
    out = np.empty((T, NF), np.float32)
    for c in range(NCORES):
        out[c * TL : (c + 1) * TL, :] = results[c]["out"]
    return out


_CACHE = {}


def get_nc():
    if "nc" not in _CACHE:
        _CACHE["nc"] = build_nc()
    return _CACHE["nc"]


def kernel(**inputs):
    from concourse import bass2jax

    nc = get_nc()
    in_maps = shard_inputs(**inputs)
    results = bass2jax.run_bass_via_pjrt(nc, in_maps, n_cores=NCORES)
    return unshard_output(results)
